# revision 4
# baseline (speedup 1.0000x reference)
"""Canny-edge BCE loss kernel for Trainium2 (8 NeuronCores, batch-parallel).

Math notes (exactness argued + verified vs the jax reference on CPU):
  * The reference binarizes to {0,255}; every Sobel magnitude is then a
    multiple of 255, so weak==strong edges and the 16-step hysteresis is an
    exact no-op.  Canny reduces to: binarize -> 3x3 Sobel -> L1 magnitude ->
    directional NMS.  Working in {0,1} scale is exact (all comparisons are
    scale invariant).
  * BCE on {0,1} edge maps takes only two values: 0 and
    C = -clip(log(max(0,1e-38)), -100).  Under XLA-CPU the fp32 denormal
    1e-38 flushes to zero, log(0)=-inf, so C == 100.0 exactly.  Hence
    loss = C * count(pred_edges != label_edges) / N.
  * NMS keep = (mag > n1) & (mag >= n2) == (mag >= max(n1+1, n2)) for the
    integer-valued magnitudes here; (n1,n2) selected by gradient direction
    with the reference's is_h -> is_v -> diag priority.

Device layout per image pair (one pred + one label image):
  one [128, 4*1028] bf16 tile; partition p, j-slice j holds image row 4p+j;
  within a slice: [pad, 512 data, pad] pixel columns, each pixel a (pred,
  label) interleaved element pair, so a +-1 pixel column shift is a +-2
  element (4-byte-aligned) offset and DVE ops keep their bf16 2x mode.
  Row +-1 shifts are free-dim +-1028 offsets for 3 of 4 slices; the
  boundary slice comes from a partition-shifted SBUF->SBUF DMA aux tile.
"""

import numpy as np

B, H, W = 32, 512, 512
NCORES = 8
PAIRS = B // NCORES          # image pairs per core
P = 128                      # SBUF partitions
J = H // P                   # rows per partition (4)
WP = W + 2                   # padded pixel columns per j-slice
SL = 2 * WP                  # elements per j-slice (pred/label interleaved)
FW = J * SL                  # tile free width (4112)
CTH = float(np.float32(0.5000001))
T22 = float(np.float32(0.4142135623730951))
T67 = float(np.float32(2.414213562373095))
N_TOT = B * H * W

_CACHE = {}


def _bce_constant() -> float:
    """-clip(log(max(0,1e-38)), -100) exactly as the jax reference computes
    it on CPU (XLA flushes the fp32 denormal -> log(0) -> -inf -> clip)."""
    try:
        import jax
        import jax.numpy as jnp

        cpu = jax.devices("cpu")[0]
        with jax.default_device(cpu):
            v = jnp.clip(jnp.log(jnp.maximum(jnp.float32(0.0), 1e-38)), -100.0, None)
            return -float(v)
    except Exception:
        return 100.0


def _build_program():
    from concourse import bacc, mybir, tile

    dt = mybir.dt
    Alu = mybir.AluOpType
    Act = mybir.ActivationFunctionType

    nc = bacc.Bacc(
        "TRN2",
        target_bir_lowering=False,
        debug=False,
        enable_asserts=False,
        num_devices=NCORES,
    )
    pred = nc.dram_tensor("pred", [PAIRS, H, W], dt.float32, kind="ExternalInput").ap()
    labels = nc.dram_tensor(
        "labels", [PAIRS, H, W], dt.float32, kind="ExternalInput"
    ).ap()
    accd = nc.dram_tensor(
        "acc_out", [P, PAIRS], dt.float32, kind="ExternalOutput"
    ).ap()

    # HBM view: partition p <- rows 4p..4p+3 (contiguous 8KB per partition)
    pred_r = pred.rearrange("b (p j) w -> b p (j w)", j=J)
    labels_r = labels.rearrange("b (p j) w -> b p (j w)", j=J)

    def v2(t):  # [P, J, SL] j-slice view
        return t[:].rearrange("p (j c) -> p j c", j=J)

    def v4(t):  # [P, J, WP, 2] pixel/lane view
        return t[:].rearrange("p (j c e) -> p j c e", j=J, e=2)

    def jv(t, j0, nj, off, cnt):  # strided data view across j-slices
        return v2(t)[:, j0 : j0 + nj, off : off + cnt]

    with tile.TileContext(nc) as tc:
        with (
            tc.tile_pool(name="xs", bufs=2) as xpool,
            tc.tile_pool(name="bb", bufs=2) as bpool,
            tc.tile_pool(name="mid", bufs=1) as mid,
            tc.tile_pool(name="aux", bufs=2) as aux,
            tc.tile_pool(name="accp", bufs=1) as accp,
        ):
            acc = accp.tile([P, PAIRS], dt.float32, tag="acc")

            for k in range(PAIRS):
                xp = xpool.tile([P, J * W], dt.float32, tag="xp")
                xl = xpool.tile([P, J * W], dt.float32, tag="xl")
                nc.sync.dma_start(xp[:], pred_r[k])
                nc.sync.dma_start(xl[:], labels_r[k])

                b = bpool.tile([P, FW], dt.bfloat16, tag="b")
                xpv = xp[:].rearrange("p (j w e) -> p j w e", j=J, e=1)
                xlv = xl[:].rearrange("p (j w e) -> p j w e", j=J, e=1)
                # binarize (exact is_ge semantics) into interleaved lanes
                nc.gpsimd.tensor_scalar(
                    v4(b)[:, :, 1 : 1 + W, 0:1], xpv, CTH, None, Alu.is_ge
                )
                nc.gpsimd.tensor_scalar(
                    v4(b)[:, :, 1 : 1 + W, 1:2], xlv, CTH, None, Alu.is_ge
                )
                # replicate-pad the outer pixel columns of every slice
                nc.gpsimd.tensor_copy(v2(b)[:, :, 0:2], v2(b)[:, :, 2:4])
                nc.gpsimd.tensor_copy(v2(b)[:, :, SL - 2 : SL], v2(b)[:, :, SL - 4 : SL - 2])

                # aux row tiles for the j-slice boundaries
                bn0 = aux.tile([P, SL], dt.bfloat16, tag="bn0")  # row 4p+4 (j=0 of p+1)
                nc.sync.dma_start(bn0[0 : P - 1, :], b[1:P, 0:SL])
                nc.sync.dma_start(bn0[P - 1 : P, :], b[P - 1 : P, 3 * SL : 4 * SL])

                # c1 = b + b_down   (row r + row r+1)
                c1 = mid.tile([P, FW], dt.bfloat16, tag="c1")
                nc.vector.tensor_tensor(
                    c1[:, 0 : 3 * SL], b[:, 0 : 3 * SL], b[:, SL : 4 * SL], Alu.add
                )
                nc.vector.tensor_tensor(
                    c1[:, 3 * SL : 4 * SL], b[:, 3 * SL : 4 * SL], bn0[:], Alu.add
                )

                c1p3 = aux.tile([P, SL], dt.bfloat16, tag="c1p3")  # c1 at row 4p-1
                nc.sync.dma_start(c1p3[1:P, :], c1[0 : P - 1, 3 * SL : 4 * SL])
                # c1 at row -1 = b(-1)+b(0) = 2*b(0)  (replicate border)
                nc.gpsimd.tensor_scalar(c1p3[0:1, :], b[0:1, 0:SL], 2.0, None, Alu.mult)

                # t = c1 + c1_up = b_up + 2b + b_down ; u = c1 - c1_up = b_down - b_up
                t = mid.tile([P, FW], dt.bfloat16, tag="t")
                u = mid.tile([P, FW], dt.bfloat16, tag="u")
                nc.vector.tensor_tensor(
                    t[:, SL:FW], c1[:, SL:FW], c1[:, 0 : 3 * SL], Alu.add
                )
                nc.vector.tensor_tensor(t[:, 0:SL], c1[:, 0:SL], c1p3[:], Alu.add)
                nc.vector.tensor_tensor(
                    u[:, SL:FW], c1[:, SL:FW], c1[:, 0 : 3 * SL], Alu.subtract
                )
                nc.vector.tensor_tensor(u[:, 0:SL], c1[:, 0:SL], c1p3[:], Alu.subtract)

                # gx = t_E - t_W  (interleaved pixel shift = +-2 elements)
                gx = mid.tile([P, FW], dt.bfloat16, tag="gx")
                DW = 2 * W  # data elements per slice
                nc.vector.tensor_tensor(
                    jv(gx, 0, J, 2, DW), jv(t, 0, J, 4, DW), jv(t, 0, J, 0, DW),
                    Alu.subtract,
                )
                # gy = u_W + 2u + u_E via r = u + u_E ; gy = r_W + r
                r = mid.tile([P, FW], dt.bfloat16, tag="r")
                nc.vector.tensor_tensor(
                    jv(r, 0, J, 0, DW + 2), jv(u, 0, J, 0, DW + 2), jv(u, 0, J, 2, DW + 2),
                    Alu.add,
                )
                gy = mid.tile([P, FW], dt.bfloat16, tag="gy")
                nc.vector.tensor_tensor(
                    jv(gy, 0, J, 2, DW), jv(r, 0, J, 0, DW), jv(r, 0, J, 2, DW), Alu.add
                )

                # ax=|gx|, ay=|gy| on ScalarE; mag = ax+ay with zeroed pads
                ax = mid.tile([P, FW], dt.bfloat16, tag="ax")
                ay = mid.tile([P, FW], dt.bfloat16, tag="ay")
                nc.scalar.activation(jv(ax, 0, J, 2, DW), jv(gx, 0, J, 2, DW), Act.Abs)
                nc.scalar.activation(jv(ay, 0, J, 2, DW), jv(gy, 0, J, 2, DW), Act.Abs)
                mag = mid.tile([P, FW], dt.bfloat16, tag="mag")
                nc.vector.tensor_tensor(
                    jv(mag, 0, J, 2, DW), jv(ax, 0, J, 2, DW), jv(ay, 0, J, 2, DW), Alu.add
                )
                nc.gpsimd.memset(jv(mag, 0, J, 0, 2), 0.0)
                nc.gpsimd.memset(jv(mag, 0, J, SL - 2, 2), 0.0)

                mn0 = aux.tile([P, SL], dt.bfloat16, tag="mn0")  # mag row 4p+4 (0 past end)
                nc.gpsimd.memset(mn0[:], 0.0)  # row 127 stays 0 (image border)
                nc.sync.dma_start(mn0[0 : P - 1, :], mag[1:P, 0:SL])
                mp3 = aux.tile([P, SL], dt.bfloat16, tag="mp3")  # mag row 4p-1 (0 before start)
                nc.sync.dma_start(mp3[1:P, :], mag[0 : P - 1, 3 * SL : 4 * SL])
                nc.gpsimd.memset(mp3[0:1, :], 0.0)

                # direction predicates
                ish = mid.tile([P, FW], dt.uint16, tag="ish")
                isv = mid.tile([P, FW], dt.uint16, tag="isv")
                nc.vector.scalar_tensor_tensor(
                    jv(ish, 0, J, 2, DW), jv(ax, 0, J, 2, DW), T22, jv(ay, 0, J, 2, DW),
                    Alu.mult, Alu.is_ge,
                )
                nc.vector.scalar_tensor_tensor(
                    jv(isv, 0, J, 2, DW), jv(ax, 0, J, 2, DW), T67, jv(ay, 0, J, 2, DW),
                    Alu.mult, Alu.is_le,
                )
                gg = mid.tile([P, FW], dt.bfloat16, tag="c1")  # reuse c1 slot
                nc.vector.tensor_tensor(
                    jv(gg, 0, J, 2, DW), jv(gx, 0, J, 2, DW), jv(gy, 0, J, 2, DW), Alu.mult
                )
                dp = mid.tile([P, FW], dt.uint16, tag="t")  # reuse t slot
                nc.vector.tensor_scalar(
                    jv(dp, 0, J, 2, DW), jv(gg, 0, J, 2, DW), 0.0, None, Alu.is_ge
                )

                # per-direction thresholds q_dir = max(n1+1, n2), then priority select
                # neighbor offsets within mag (elements): E=+2, W=-2 relative to out
                def stt_q(dst, n1_ap, n2_ap):
                    nc.vector.scalar_tensor_tensor(
                        dst, n1_ap, 1.0, n2_ap, Alu.add, Alu.max
                    )

                q = mid.tile([P, FW], dt.bfloat16, tag="u")  # reuse u slot
                # q := q_d2 (n1=NE, n2=SW)
                stt_q(jv(q, 0, 1, 2, DW), mp3[:, 4 : 4 + DW], jv(mag, 1, 1, 0, DW))
                stt_q(jv(q, 1, 2, 2, DW), jv(mag, 0, 2, 4, DW), jv(mag, 2, 2, 0, DW))
                stt_q(jv(q, 3, 1, 2, DW), jv(mag, 2, 1, 4, DW), mn0[:, 0:DW])
                # q_d1 (n1=NW, n2=SE)
                qd1 = mid.tile([P, FW], dt.bfloat16, tag="gx")  # reuse gx slot
                stt_q(jv(qd1, 0, 1, 2, DW), mp3[:, 0:DW], jv(mag, 1, 1, 4, DW))
                stt_q(jv(qd1, 1, 2, 2, DW), jv(mag, 0, 2, 0, DW), jv(mag, 2, 2, 4, DW))
                stt_q(jv(qd1, 3, 1, 2, DW), jv(mag, 2, 1, 0, DW), mn0[:, 4 : 4 + DW])
                # q_v (n1=N, n2=S)
                qv = mid.tile([P, FW], dt.bfloat16, tag="ax")  # reuse ax slot
                stt_q(jv(qv, 0, 1, 2, DW), mp3[:, 2 : 2 + DW], jv(mag, 1, 1, 2, DW))
                stt_q(jv(qv, 1, 2, 2, DW), jv(mag, 0, 2, 2, DW), jv(mag, 2, 2, 2, DW))
                stt_q(jv(qv, 3, 1, 2, DW), jv(mag, 2, 1, 2, DW), mn0[:, 2 : 2 + DW])
                # q_h (n1=W, n2=E)
                qh = mid.tile([P, FW], dt.bfloat16, tag="ay")  # reuse ay slot
                stt_q(jv(qh, 0, J, 2, DW), jv(mag, 0, J, 0, DW), jv(mag, 0, J, 4, DW))

                nc.vector.copy_predicated(
                    jv(q, 0, J, 2, DW), jv(dp, 0, J, 2, DW), jv(qd1, 0, J, 2, DW)
                )
                nc.vector.copy_predicated(
                    jv(q, 0, J, 2, DW), jv(isv, 0, J, 2, DW), jv(qv, 0, J, 2, DW)
                )
                nc.vector.copy_predicated(
                    jv(q, 0, J, 2, DW), jv(ish, 0, J, 2, DW), jv(qh, 0, J, 2, DW)
                )

                keep = mid.tile([P, FW], dt.bfloat16, tag="r")  # reuse r slot
                nc.vector.tensor_tensor(
                    jv(keep, 0, J, 2, DW), jv(mag, 0, J, 2, DW), jv(q, 0, J, 2, DW),
                    Alu.is_ge,
                )

                # d = (keep_pred != keep_label), accumulated count per partition
                d = mid.tile([P, J * W], dt.bfloat16, tag="d")
                dv = d[:].rearrange("p (j w e) -> p j w e", j=J, e=1)
                nc.vector.scalar_tensor_tensor(
                    dv,
                    v4(keep)[:, :, 1 : 1 + W, 0:1],
                    1.0,
                    v4(keep)[:, :, 1 : 1 + W, 1:2],
                    Alu.mult,
                    Alu.not_equal,
                    accum_out=acc[:, k : k + 1],
                )

            nc.sync.dma_start(accd[:], acc[:])

    nc.compile()
    return nc


def _get_program():
    if "nc" not in _CACHE:
        _CACHE["nc"] = _build_program()
    return _CACHE["nc"]


def kernel(pred: np.ndarray, labels: np.ndarray) -> np.ndarray:
    from concourse import bass_utils

    pred = np.asarray(pred).reshape(B, H, W).astype(np.float32, copy=False)
    labels = np.asarray(labels).reshape(B, H, W).astype(np.float32, copy=False)

    nc = _get_program()
    in_maps = []
    for c in range(NCORES):
        sl = slice(c * PAIRS, (c + 1) * PAIRS)
        in_maps.append(
            {
                "pred": np.ascontiguousarray(pred[sl]),
                "labels": np.ascontiguousarray(labels[sl]),
            }
        )
    res = bass_utils.run_bass_kernel_spmd(nc, in_maps, core_ids=list(range(NCORES)))
    k_total = sum(float(r["acc_out"].astype(np.float64).sum()) for r in res.results)
    loss = np.float32(_bce_constant() * k_total / float(N_TOT))
    return np.array(loss, dtype=np.float32)


# revision 11
# speedup vs baseline: 2.2206x; 2.2206x over previous
"""Canny-edge BCE loss kernel for Trainium2 (8 NeuronCores, batch-parallel).

Math notes (exactness argued + verified vs the jax reference on CPU):
  * The reference binarizes to {0,255}; every Sobel magnitude is then a
    multiple of 255, so weak==strong edges and the 16-step hysteresis is an
    exact no-op.  Canny reduces to: binarize -> 3x3 Sobel -> L1 magnitude ->
    directional NMS.  Working in {0,1} scale is exact (all comparisons are
    scale invariant).
  * BCE on {0,1} edge maps takes only two values: 0 and
    C = -clip(log(max(0,1e-38)), -100).  Under XLA-CPU the fp32 denormal
    1e-38 flushes to zero, log(0)=-inf, so C == 100.0 exactly.  Hence
    loss = C * count(pred_edges != label_edges) / N.
  * NMS keep = (mag > n1) & (mag >= n2) == (mag >= max(n1+1, n2)) for the
    integer-valued magnitudes here; (n1,n2) selected by gradient direction
    with the reference's is_h -> is_v -> diag priority.

Device layout per image pair (one pred + one label image):
  one [128, 4*1028] bf16 tile; partition p, j-slice j holds image row 4p+j;
  within a slice: [pad, 512 data, pad] pixel columns, each pixel a (pred,
  label) interleaved element pair, so a +-1 pixel column shift is a +-2
  element (4-byte-aligned) offset.  Row +-1 shifts are free-dim +-1028
  offsets for 3 of 4 slices; the boundary slice is a partition-shifted
  copy produced on the TensorEngine (eye(128,k=+-1) matmul, which also
  zeroes the image border rows for free).  Elementwise ops run on
  contiguous 1D ranges (strided DVE ops measure ~3-4x slower); garbage in
  pad columns is harmless and mag's pads are re-zeroed explicitly.
"""

import numpy as np

B, H, W = 32, 512, 512
NCORES = 8
PAIRS = B // NCORES          # image pairs per core
P = 128                      # SBUF partitions
J = H // P                   # rows per partition (4)
WP = W + 2                   # padded pixel columns per j-slice
SL = 2 * WP                  # elements per j-slice (pred/label interleaved)
FW = J * SL                  # tile free width (4112)
SLK = SL + 4                 # aux row tiles carry 2-element slack each side
HSL = SL // 2                # matmul free-dim half (PSUM bank limit)
CTH = float(np.float32(0.5000001))
T22 = float(np.float32(0.4142135623730951))
T67 = float(np.float32(2.414213562373095))
N_TOT = B * H * W

_CACHE = {}


def _bce_constant() -> float:
    """-clip(log(max(0,1e-38)), -100) exactly as the jax reference computes
    it on CPU (XLA flushes the fp32 denormal -> log(0) -> -inf -> clip)."""
    try:
        import jax
        import jax.numpy as jnp

        cpu = jax.devices("cpu")[0]
        with jax.default_device(cpu):
            v = jnp.clip(jnp.log(jnp.maximum(jnp.float32(0.0), 1e-38)), -100.0, None)
            return -float(v)
    except Exception:
        return 100.0


def _shift_mats() -> np.ndarray:
    import ml_dtypes

    m = np.zeros((P, 2 * P), dtype=np.float32)
    m[:, 0:P] = np.eye(P, k=-1)   # SUP: out[m] = in[m+1], out[127] = 0
    m[:, P : 2 * P] = np.eye(P, k=1)  # SDN: out[m] = in[m-1], out[0] = 0
    return m.astype(ml_dtypes.bfloat16)


def _build_program():
    from concourse import bacc, mybir, tile

    dt = mybir.dt
    Alu = mybir.AluOpType
    Act = mybir.ActivationFunctionType

    nc = bacc.Bacc(
        "TRN2",
        target_bir_lowering=False,
        debug=False,
        enable_asserts=False,
        num_devices=NCORES,
    )
    pred = nc.dram_tensor("pred", [PAIRS, H, W], dt.float32, kind="ExternalInput").ap()
    labels = nc.dram_tensor(
        "labels", [PAIRS, H, W], dt.float32, kind="ExternalInput"
    ).ap()
    shifts = nc.dram_tensor(
        "shifts", [P, 2 * P], dt.bfloat16, kind="ExternalInput"
    ).ap()
    accd = nc.dram_tensor(
        "acc_out", [P, PAIRS], dt.float32, kind="ExternalOutput"
    ).ap()

    # HBM view: partition p <- rows 4p..4p+3 (contiguous 8KB per partition)
    pred_r = pred.rearrange("b (p j) w -> b p (j w)", j=J)
    labels_r = labels.rearrange("b (p j) w -> b p (j w)", j=J)

    def v2(t):  # [P, J, SL] j-slice view
        return t[:].rearrange("p (j c) -> p j c", j=J)

    def v4(t):  # [P, J, WP, 2] pixel/lane view
        return t[:].rearrange("p (j c e) -> p j c e", j=J, e=2)

    with tile.TileContext(nc) as tc:
        with (
            tc.tile_pool(name="xs", bufs=2) as xpool,
            tc.tile_pool(name="bb", bufs=2) as bpool,
            tc.tile_pool(name="mid", bufs=1) as mid,
            tc.tile_pool(name="aux", bufs=2) as aux,
            tc.tile_pool(name="cst", bufs=1) as cpool,
            tc.tile_pool(name="ps", bufs=4, space="PSUM") as psum,
            tc.tile_pool(name="accp", bufs=1) as accp,
        ):
            acc = accp.tile([P, PAIRS], dt.float32, tag="acc")
            shm = cpool.tile([P, 2 * P], dt.bfloat16, tag="shm")
            nc.sync.dma_start(shm[:], shifts[:])
            sup = shm[:, 0:P]
            sdn = shm[:, P : 2 * P]

            def shift_rows(dst, dst_off, src, src_off, mat):
                """dst[p, dst_off+2+i] = src[p+-1, src_off+2+i], i in [0,1024):
                partition shift of a slice's data elements via two
                [128,128]@[128,512] matmuls (border row -> 0).  Pad columns
                are NOT produced; callers fill them."""
                for h in range(2):
                    ps = psum.tile([P, W], dt.float32, tag="ps")
                    lo = src_off + 2 + h * W
                    nc.tensor.matmul(ps[:], mat, src[:, lo : lo + W])
                    dlo = dst_off + 2 + h * W
                    nc.scalar.activation(dst[:, dlo : dlo + W], ps[:], Act.Copy)

            for k in range(PAIRS):
                xp = xpool.tile([P, J * W], dt.float32, tag="xp")
                xl = xpool.tile([P, J * W], dt.float32, tag="xl")
                nc.sync.dma_start(xp[:], pred_r[k])
                nc.sync.dma_start(xl[:], labels_r[k])

                b = bpool.tile([P, FW], dt.bfloat16, tag="b")
                xpv = xp[:].rearrange("p (j w e) -> p j w e", j=J, e=1)
                xlv = xl[:].rearrange("p (j w e) -> p j w e", j=J, e=1)
                # binarize (exact is_ge semantics) into interleaved lanes
                nc.vector.tensor_scalar(
                    v4(b)[:, :, 1 : 1 + W, 0:1], xpv, CTH, None, Alu.is_ge
                )
                nc.vector.tensor_scalar(
                    v4(b)[:, :, 1 : 1 + W, 1:2], xlv, CTH, None, Alu.is_ge
                )
                # replicate-pad the outer pixel columns of every slice
                nc.vector.tensor_copy(v2(b)[:, :, 0:2], v2(b)[:, :, 2:4])
                nc.vector.tensor_copy(
                    v2(b)[:, :, SL - 2 : SL], v2(b)[:, :, SL - 4 : SL - 2]
                )

                # bn0[p] = b[p+1, j=0] (row 4p+4); row 127 = replicate row 511
                bn0 = aux.tile([P, SL], dt.bfloat16, tag="bn0")
                shift_rows(bn0, 0, b, 0, sup)
                # pad columns replicate the edge data columns (as in b itself)
                nc.vector.tensor_copy(bn0[:, 0:2], bn0[:, 2:4])
                nc.vector.tensor_copy(bn0[:, SL - 2 : SL], bn0[:, SL - 4 : SL - 2])
                nc.sync.dma_start(bn0[P - 1 : P, :], b[P - 1 : P, 3 * SL : 4 * SL])

                # c1 = b + b_down   (row r + row r+1)
                c1 = mid.tile([P, FW], dt.bfloat16, tag="c1")
                nc.vector.tensor_tensor(
                    c1[:, 0 : 3 * SL], b[:, 0 : 3 * SL], b[:, SL : 4 * SL], Alu.add
                )
                nc.vector.tensor_tensor(
                    c1[:, 3 * SL : 4 * SL], b[:, 3 * SL : 4 * SL], bn0[:], Alu.add
                )

                # c1p3[p] = c1[p-1, j=3] (c1 at row 4p-1); row 0 = 2*b(row 0)
                c1p3 = aux.tile([P, SL], dt.bfloat16, tag="c1p3")
                shift_rows(c1p3, 0, c1, 3 * SL, sdn)
                nc.vector.tensor_copy(c1p3[:, 0:2], c1p3[:, 2:4])
                nc.vector.tensor_copy(c1p3[:, SL - 2 : SL], c1p3[:, SL - 4 : SL - 2])
                nc.scalar.mul(c1p3[0:1, :], b[0:1, 0:SL], 2.0)

                # t = c1 + c1_up = b_up + 2b + b_down ; u = c1 - c1_up
                t = mid.tile([P, FW], dt.bfloat16, tag="t")
                u = mid.tile([P, FW], dt.bfloat16, tag="u")
                nc.vector.tensor_tensor(
                    t[:, SL:FW], c1[:, SL:FW], c1[:, 0 : 3 * SL], Alu.add
                )
                nc.vector.tensor_tensor(t[:, 0:SL], c1[:, 0:SL], c1p3[:], Alu.add)
                nc.vector.tensor_tensor(
                    u[:, SL:FW], c1[:, SL:FW], c1[:, 0 : 3 * SL], Alu.subtract
                )
                nc.vector.tensor_tensor(u[:, 0:SL], c1[:, 0:SL], c1p3[:], Alu.subtract)

                # contiguous full-width x-shift ops (slice crossings only
                # corrupt pad columns; data columns read correct pads)
                gx = mid.tile([P, FW], dt.bfloat16, tag="gx")
                nc.vector.tensor_tensor(
                    gx[:, 2 : FW - 2], t[:, 4:FW], t[:, 0 : FW - 4], Alu.subtract
                )
                r = mid.tile([P, FW], dt.bfloat16, tag="r")
                nc.vector.tensor_tensor(
                    r[:, 0 : FW - 2], u[:, 0 : FW - 2], u[:, 2:FW], Alu.add
                )
                gy = mid.tile([P, FW], dt.bfloat16, tag="gy")
                nc.vector.tensor_tensor(
                    gy[:, 2 : FW - 2], r[:, 0 : FW - 4], r[:, 2 : FW - 2], Alu.add
                )

                ax = mid.tile([P, FW], dt.bfloat16, tag="ax")
                ay = mid.tile([P, FW], dt.bfloat16, tag="ay")
                nc.scalar.activation(ax[:, 2 : FW - 2], gx[:, 2 : FW - 2], Act.Abs)
                nc.scalar.activation(ay[:, 2 : FW - 2], gy[:, 2 : FW - 2], Act.Abs)
                mag = mid.tile([P, FW], dt.bfloat16, tag="mag")
                nc.vector.tensor_tensor(
                    mag[:, 2 : FW - 2], ax[:, 2 : FW - 2], ay[:, 2 : FW - 2], Alu.add
                )
                # NMS uses a zero border: zero every pad column (also covers
                # the tile's first/last two elements)
                nc.vector.memset(v2(mag)[:, :, 0:2], 0.0)
                nc.vector.memset(v2(mag)[:, :, SL - 2 : SL], 0.0)

                # mag row shifts via TensorE; border rows are zero (exact)
                mn0 = aux.tile([P, SLK], dt.bfloat16, tag="mn0")  # mag row 4p+4
                mp3 = aux.tile([P, SLK], dt.bfloat16, tag="mp3")  # mag row 4p-1
                shift_rows(mn0, 2, mag, 0, sup)
                shift_rows(mp3, 2, mag, 3 * SL, sdn)
                for aux_t in (mn0, mp3):
                    # slack + the slice's own zero pad columns
                    nc.vector.memset(aux_t[:, 0:4], 0.0)
                    nc.vector.memset(aux_t[:, SLK - 4 : SLK], 0.0)

                # direction predicates (contiguous, data region only)
                ish = mid.tile([P, FW], dt.uint16, tag="ish")
                isv = mid.tile([P, FW], dt.uint16, tag="isv")
                nc.vector.scalar_tensor_tensor(
                    ish[:, 2 : FW - 2], ax[:, 2 : FW - 2], T22, ay[:, 2 : FW - 2],
                    Alu.mult, Alu.is_ge,
                )
                nc.vector.scalar_tensor_tensor(
                    isv[:, 2 : FW - 2], ax[:, 2 : FW - 2], T67, ay[:, 2 : FW - 2],
                    Alu.mult, Alu.is_le,
                )
                gg = mid.tile([P, FW], dt.bfloat16, tag="c1")  # reuse c1 slot
                nc.vector.tensor_tensor(
                    gg[:, 2 : FW - 2], gx[:, 2 : FW - 2], gy[:, 2 : FW - 2], Alu.mult
                )
                dp = mid.tile([P, FW], dt.uint16, tag="t")  # reuse t slot
                nc.vector.tensor_scalar(
                    dp[:, 2 : FW - 2], gg[:, 2 : FW - 2], 0.0, None, Alu.is_ge
                )

                # q_dir = max(n1+1, n2), all contiguous ops; aux tiles hold
                # the row-shifted slice at offset 2 with slack both sides
                def stt_q(dst, n1_ap, n2_ap):
                    nc.vector.scalar_tensor_tensor(
                        dst, n1_ap, 1.0, n2_ap, Alu.add, Alu.max
                    )

                q = mid.tile([P, FW], dt.bfloat16, tag="u")  # reuse u slot
                # q := q_d2 (n1=NE=row-1,col+1 ; n2=SW=row+1,col-1)
                stt_q(q[:, 0:SL], mp3[:, 4 : 4 + SL], mag[:, SL - 2 : 2 * SL - 2])
                stt_q(
                    q[:, SL : 3 * SL],
                    mag[:, 2 : 2 * SL + 2],
                    mag[:, 2 * SL - 2 : 4 * SL - 2],
                )
                stt_q(q[:, 3 * SL : FW], mag[:, 2 * SL + 2 : 3 * SL + 2], mn0[:, 0:SL])
                # q_d1 (n1=NW=row-1,col-1 ; n2=SE=row+1,col+1)
                qd1 = mid.tile([P, FW], dt.bfloat16, tag="gx")  # reuse gx slot
                stt_q(qd1[:, 0 : SL + 2], mp3[:, 0 : SL + 2], mag[:, SL + 2 : 2 * SL + 4])
                stt_q(
                    qd1[:, SL + 2 : 3 * SL - 2],
                    mag[:, 0 : 2 * SL - 4],
                    mag[:, 2 * SL + 4 : 4 * SL],
                )
                stt_q(
                    qd1[:, 3 * SL - 2 : FW],
                    mag[:, 2 * SL - 4 : 3 * SL - 2],
                    mn0[:, 2 : SL + 4],
                )
                # q_v (n1=N=row-1 ; n2=S=row+1)
                qv = mid.tile([P, FW], dt.bfloat16, tag="ax")  # reuse ax slot
                stt_q(qv[:, 0:SL], mp3[:, 2 : 2 + SL], mag[:, SL : 2 * SL])
                stt_q(qv[:, SL : 3 * SL], mag[:, 0 : 2 * SL], mag[:, 2 * SL : FW])
                stt_q(qv[:, 3 * SL : FW], mag[:, 2 * SL : 3 * SL], mn0[:, 2 : 2 + SL])
                # q_h (n1=W=col-1 ; n2=E=col+1)
                qh = mid.tile([P, FW], dt.bfloat16, tag="ay")  # reuse ay slot
                stt_q(qh[:, 2 : FW - 2], mag[:, 0 : FW - 4], mag[:, 4:FW])

                # priority select: d2 -> d1 (diag_pos) -> v (is_v) -> h (is_h)
                nc.vector.copy_predicated(
                    q[:, 2 : FW - 2], dp[:, 2 : FW - 2], qd1[:, 2 : FW - 2]
                )
                nc.vector.copy_predicated(
                    q[:, 2 : FW - 2], isv[:, 2 : FW - 2], qv[:, 2 : FW - 2]
                )
                nc.vector.copy_predicated(
                    q[:, 2 : FW - 2], ish[:, 2 : FW - 2], qh[:, 2 : FW - 2]
                )

                keep = mid.tile([P, FW], dt.bfloat16, tag="r")  # reuse r slot
                nc.vector.tensor_tensor(
                    keep[:, 2 : FW - 2], mag[:, 2 : FW - 2], q[:, 2 : FW - 2], Alu.is_ge
                )

                # d = (keep_pred != keep_label), accumulated count per partition
                d = mid.tile([P, J * W], dt.bfloat16, tag="d")
                dv = d[:].rearrange("p (j w e) -> p j w e", j=J, e=1)
                nc.vector.scalar_tensor_tensor(
                    dv,
                    v4(keep)[:, :, 1 : 1 + W, 0:1],
                    1.0,
                    v4(keep)[:, :, 1 : 1 + W, 1:2],
                    Alu.mult,
                    Alu.not_equal,
                    accum_out=acc[:, k : k + 1],
                )

            nc.sync.dma_start(accd[:], acc[:])

    nc.compile()
    return nc


def _get_program():
    if "nc" not in _CACHE:
        _CACHE["nc"] = _build_program()
    return _CACHE["nc"]


def kernel(pred: np.ndarray, labels: np.ndarray) -> np.ndarray:
    from concourse import bass_utils

    pred = np.asarray(pred).reshape(B, H, W).astype(np.float32, copy=False)
    labels = np.asarray(labels).reshape(B, H, W).astype(np.float32, copy=False)

    nc = _get_program()
    shifts = _shift_mats()
    in_maps = []
    for c in range(NCORES):
        sl = slice(c * PAIRS, (c + 1) * PAIRS)
        in_maps.append(
            {
                "pred": np.ascontiguousarray(pred[sl]),
                "labels": np.ascontiguousarray(labels[sl]),
                "shifts": shifts,
            }
        )
    res = bass_utils.run_bass_kernel_spmd(nc, in_maps, core_ids=list(range(NCORES)))
    k_total = sum(float(r["acc_out"].astype(np.float64).sum()) for r in res.results)
    loss = np.float32(_bce_constant() * k_total / float(N_TOT))
    return np.array(loss, dtype=np.float32)


# revision 19
# speedup vs baseline: 2.6224x; 1.1809x over previous
"""Canny-edge BCE loss kernel for Trainium2 (8 NeuronCores, batch-parallel).

Math notes (exactness argued + verified vs the jax reference on CPU):
  * The reference binarizes to {0,255}; every Sobel magnitude is then a
    multiple of 255, so weak==strong edges and the 16-step hysteresis is an
    exact no-op.  Canny reduces to: binarize -> 3x3 Sobel -> L1 magnitude ->
    directional NMS.  Working in {0,1} scale is exact (all comparisons are
    scale invariant).
  * BCE on {0,1} edge maps takes only two values: 0 and
    C = -clip(log(max(0,1e-38)), -100).  Under XLA-CPU the fp32 denormal
    1e-38 flushes to zero, log(0)=-inf, so C == 100.0 exactly.  Hence
    loss = C * count(pred_edges != label_edges) / N.
  * NMS keep = (mag > n1) & (mag >= n2) == (mag >= max(n1+1, n2)) for the
    integer-valued magnitudes here; (n1,n2) selected by gradient direction
    with the reference's is_h -> is_v -> diag priority.

Device layout per image pair (one pred + one label image):
  one [128, 4*1028] bf16 tile; partition p, j-slice j holds image row 4p+j;
  within a slice: [pad, 512 data, pad] pixel columns, each pixel a (pred,
  label) interleaved element pair, so a +-1 pixel column shift is a +-2
  element (4-byte-aligned) offset.  Row +-1 shifts are free-dim +-1028
  offsets for 3 of 4 slices; the boundary slice is a partition-shifted
  copy produced on the TensorEngine (eye(128,k=+-1) matmul, which also
  zeroes the image border rows for free).  Elementwise ops run on
  contiguous 1D ranges (strided DVE ops measure ~3-4x slower); garbage in
  pad columns is harmless and mag's pads are re-zeroed explicitly.
"""

import numpy as np

B, H, W = 32, 512, 512
NCORES = 8
PAIRS = B // NCORES          # image pairs per core
P = 128                      # SBUF partitions
J = H // P                   # rows per partition (4)
WP = W + 2                   # padded pixel columns per j-slice
SL = 2 * WP                  # elements per j-slice (pred/label interleaved)
FW = J * SL                  # tile free width (4112)
SLK = SL + 4                 # aux row tiles carry 2-element slack each side
HSL = SL // 2                # matmul free-dim half (PSUM bank limit)
CTH = float(np.float32(0.5000001))
T22 = float(np.float32(0.4142135623730951))
T67 = float(np.float32(2.414213562373095))
N_TOT = B * H * W

_CACHE = {}


def _bce_constant() -> float:
    """-clip(log(max(0,1e-38)), -100) exactly as the jax reference computes
    it on CPU (XLA flushes the fp32 denormal -> log(0) -> -inf -> clip)."""
    try:
        import jax
        import jax.numpy as jnp

        cpu = jax.devices("cpu")[0]
        with jax.default_device(cpu):
            v = jnp.clip(jnp.log(jnp.maximum(jnp.float32(0.0), 1e-38)), -100.0, None)
            return -float(v)
    except Exception:
        return 100.0


def _shift_mats() -> np.ndarray:
    import ml_dtypes

    m = np.zeros((P, 4 * P), dtype=np.float32)
    m[:, 0:P] = np.eye(P, k=-1)       # SUP: out[m] = in[m+1], out[127] = 0
    m[:, P : 2 * P] = np.eye(P, k=1)  # SDN: out[m] = in[m-1], out[0] = 0
    m[P - 1, 3 * P - 1] = 1.0         # E127: out[127] = in[127], else 0
    m[0, 3 * P] = 2.0                 # E0x2: out[0] = 2*in[0], else 0
    return m.astype(ml_dtypes.bfloat16)


def _build_program():
    from concourse import bacc, mybir, tile

    dt = mybir.dt
    Alu = mybir.AluOpType
    Act = mybir.ActivationFunctionType

    nc = bacc.Bacc(
        "TRN2",
        target_bir_lowering=False,
        debug=False,
        enable_asserts=False,
        num_devices=NCORES,
    )
    pred = nc.dram_tensor("pred", [PAIRS, H, W], dt.float32, kind="ExternalInput").ap()
    labels = nc.dram_tensor(
        "labels", [PAIRS, H, W], dt.float32, kind="ExternalInput"
    ).ap()
    shifts = nc.dram_tensor(
        "shifts", [P, 4 * P], dt.bfloat16, kind="ExternalInput"
    ).ap()
    accd = nc.dram_tensor(
        "acc_out", [P, PAIRS], dt.float32, kind="ExternalOutput"
    ).ap()

    # HBM view: partition p <- rows 4p..4p+3 (contiguous 8KB per partition)
    pred_r = pred.rearrange("b (p j) w -> b p (j w)", j=J)
    labels_r = labels.rearrange("b (p j) w -> b p (j w)", j=J)

    def v2(t):  # [P, J, SL] j-slice view
        return t[:].rearrange("p (j c) -> p j c", j=J)

    def v4(t):  # [P, J, WP, 2] pixel/lane view
        return t[:].rearrange("p (j c e) -> p j c e", j=J, e=2)

    with tile.TileContext(nc) as tc:
        with (
            tc.tile_pool(name="xs", bufs=2) as xpool,
            tc.tile_pool(name="bb", bufs=2) as bpool,
            tc.tile_pool(name="mid", bufs=1) as mid,
            tc.tile_pool(name="aux", bufs=2) as aux,
            tc.tile_pool(name="cst", bufs=1) as cpool,
            tc.tile_pool(name="ps", bufs=4, space="PSUM") as psum,
            tc.tile_pool(name="accp", bufs=1) as accp,
        ):
            acc = accp.tile([P, PAIRS], dt.float32, tag="acc")
            shm = cpool.tile([P, 4 * P], dt.bfloat16, tag="shm")
            nc.sync.dma_start(shm[:], shifts[:])
            sup = shm[:, 0:P]
            sdn = shm[:, P : 2 * P]
            e127 = shm[:, 2 * P : 3 * P]
            e0x2 = shm[:, 3 * P : 4 * P]

            def shift_rows(dst, dst_off, src, src_off, mat, fix=None):
                """dst[p, dst_off+2+i] = src[p+-1, src_off+2+i], i in [0,1024):
                partition shift of a slice's data elements via two
                [128,128]@[128,512] matmuls (border row -> 0).  `fix` is an
                optional (matrix, src2, src2_off) accumulated on top to
                patch the border row.  Pad columns are NOT produced."""
                for h in range(2):
                    ps = psum.tile([P, W], dt.float32, tag="ps")
                    lo = src_off + 2 + h * W
                    nc.tensor.matmul(
                        ps[:], mat, src[:, lo : lo + W],
                        start=True, stop=fix is None,
                    )
                    if fix is not None:
                        mat2, src2, s2off = fix
                        lo2 = s2off + 2 + h * W
                        nc.tensor.matmul(
                            ps[:], mat2, src2[:, lo2 : lo2 + W],
                            start=False, stop=True,
                        )
                    dlo = dst_off + 2 + h * W
                    nc.scalar.activation(dst[:, dlo : dlo + W], ps[:], Act.Copy)

            for k in range(PAIRS):
                xp = xpool.tile([P, J * W], dt.float32, tag="xp")
                xl = xpool.tile([P, J * W], dt.float32, tag="xl")
                nc.sync.dma_start(xp[:], pred_r[k])
                nc.sync.dma_start(xl[:], labels_r[k])

                b = bpool.tile([P, FW], dt.bfloat16, tag="b")
                xpv = xp[:].rearrange("p (j w e) -> p j w e", j=J, e=1)
                xlv = xl[:].rearrange("p (j w e) -> p j w e", j=J, e=1)
                # binarize (exact is_ge semantics) into interleaved lanes
                nc.vector.tensor_scalar(
                    v4(b)[:, :, 1 : 1 + W, 0:1], xpv, CTH, None, Alu.is_ge
                )
                nc.vector.tensor_scalar(
                    v4(b)[:, :, 1 : 1 + W, 1:2], xlv, CTH, None, Alu.is_ge
                )
                # replicate-pad the outer pixel columns of every slice
                nc.vector.tensor_copy(v2(b)[:, :, 0:2], v2(b)[:, :, 2:4])
                nc.vector.tensor_copy(
                    v2(b)[:, :, SL - 2 : SL], v2(b)[:, :, SL - 4 : SL - 2]
                )

                # bn0[p] = b[p+1, j=0] (row 4p+4); row 127 = replicate row 511
                bn0 = aux.tile([P, SL], dt.bfloat16, tag="bn0")
                shift_rows(bn0, 0, b, 0, sup, fix=(e127, b, 3 * SL))
                # pad columns replicate the edge data columns (as in b itself)
                nc.vector.tensor_copy(bn0[:, 0:2], bn0[:, 2:4])
                nc.vector.tensor_copy(bn0[:, SL - 2 : SL], bn0[:, SL - 4 : SL - 2])

                # c1 = b + b_down   (row r + row r+1)
                c1 = mid.tile([P, FW], dt.bfloat16, tag="c1")
                nc.vector.tensor_tensor(
                    c1[:, 0 : 3 * SL], b[:, 0 : 3 * SL], b[:, SL : 4 * SL], Alu.add
                )
                nc.vector.tensor_tensor(
                    c1[:, 3 * SL : 4 * SL], b[:, 3 * SL : 4 * SL], bn0[:], Alu.add
                )

                # c1p3[p] = c1[p-1, j=3] (c1 at row 4p-1); row 0 = 2*b(row 0)
                c1p3 = aux.tile([P, SL], dt.bfloat16, tag="c1p3")
                shift_rows(c1p3, 0, c1, 3 * SL, sdn, fix=(e0x2, b, 0))
                nc.vector.tensor_copy(c1p3[:, 0:2], c1p3[:, 2:4])
                nc.vector.tensor_copy(c1p3[:, SL - 2 : SL], c1p3[:, SL - 4 : SL - 2])

                # t = c1 + c1_up = b_up + 2b + b_down ; u = c1 - c1_up
                t = mid.tile([P, FW], dt.bfloat16, tag="t")
                u = mid.tile([P, FW], dt.bfloat16, tag="u")
                nc.vector.tensor_tensor(
                    t[:, SL:FW], c1[:, SL:FW], c1[:, 0 : 3 * SL], Alu.add
                )
                nc.vector.tensor_tensor(t[:, 0:SL], c1[:, 0:SL], c1p3[:], Alu.add)
                nc.vector.tensor_tensor(
                    u[:, SL:FW], c1[:, SL:FW], c1[:, 0 : 3 * SL], Alu.subtract
                )
                nc.vector.tensor_tensor(u[:, 0:SL], c1[:, 0:SL], c1p3[:], Alu.subtract)

                # contiguous full-width x-shift ops (slice crossings only
                # corrupt pad columns; data columns read correct pads)
                gx = mid.tile([P, FW], dt.bfloat16, tag="gx")
                nc.vector.tensor_tensor(
                    gx[:, 2 : FW - 2], t[:, 4:FW], t[:, 0 : FW - 4], Alu.subtract
                )
                r = mid.tile([P, FW], dt.bfloat16, tag="r")
                nc.vector.tensor_tensor(
                    r[:, 0 : FW - 2], u[:, 0 : FW - 2], u[:, 2:FW], Alu.add
                )
                gy = mid.tile([P, FW], dt.bfloat16, tag="gy")
                nc.vector.tensor_tensor(
                    gy[:, 2 : FW - 2], r[:, 0 : FW - 4], r[:, 2 : FW - 2], Alu.add
                )

                ax = mid.tile([P, FW], dt.bfloat16, tag="ax")
                ay = mid.tile([P, FW], dt.bfloat16, tag="ay")
                nc.scalar.activation(ax[:, 2 : FW - 2], gx[:, 2 : FW - 2], Act.Abs)
                nc.scalar.activation(ay[:, 2 : FW - 2], gy[:, 2 : FW - 2], Act.Abs)
                mag = mid.tile([P, FW], dt.bfloat16, tag="mag")
                nc.vector.tensor_tensor(
                    mag[:, 2 : FW - 2], ax[:, 2 : FW - 2], ay[:, 2 : FW - 2], Alu.add
                )
                # NMS uses a zero border: zero every pad column (also covers
                # the tile's first/last two elements)
                nc.vector.memset(v2(mag)[:, :, 0:2], 0.0)
                nc.vector.memset(v2(mag)[:, :, SL - 2 : SL], 0.0)

                # mag row shifts via TensorE; border rows are zero (exact)
                mn0 = aux.tile([P, SLK], dt.bfloat16, tag="mn0")  # mag row 4p+4
                mp3 = aux.tile([P, SLK], dt.bfloat16, tag="mp3")  # mag row 4p-1
                shift_rows(mn0, 2, mag, 0, sup)
                shift_rows(mp3, 2, mag, 3 * SL, sdn)
                for aux_t in (mn0, mp3):
                    # slack + the slice's own zero pad columns
                    nc.vector.memset(aux_t[:, 0:4], 0.0)
                    nc.vector.memset(aux_t[:, SLK - 4 : SLK], 0.0)

                # direction predicates (contiguous, data region only).
                gg = mid.tile([P, FW], dt.bfloat16, tag="c1")  # reuse c1 slot
                nc.vector.tensor_tensor(
                    gg[:, 2 : FW - 2], gx[:, 2 : FW - 2], gy[:, 2 : FW - 2], Alu.mult
                )
                dp = mid.tile([P, FW], dt.uint16, tag="t")  # reuse t slot
                nc.vector.tensor_scalar(
                    dp[:, 2 : FW - 2], gg[:, 2 : FW - 2], 0.0, None, Alu.is_ge
                )
                # scalar_tensor_tensor only runs at 1x on the DVE, so do the
                # TAN* scaling on ScalarE and keep the DVE ops plain TT (2x).
                axs = mid.tile([P, FW], dt.bfloat16, tag="gy")  # reuse gy slot
                nc.scalar.mul(axs[:, 2 : FW - 2], ax[:, 2 : FW - 2], T22)
                ish = mid.tile([P, FW], dt.uint16, tag="ish")
                isv = mid.tile([P, FW], dt.uint16, tag="isv")
                nc.vector.tensor_tensor(
                    ish[:, 2 : FW - 2], axs[:, 2 : FW - 2], ay[:, 2 : FW - 2], Alu.is_ge
                )
                axs2 = mid.tile([P, FW], dt.bfloat16, tag="d")  # reuse d slot
                nc.scalar.mul(axs2[:, 2 : FW - 2], ax[:, 2 : FW - 2], T67)
                nc.vector.tensor_tensor(
                    isv[:, 2 : FW - 2], axs2[:, 2 : FW - 2], ay[:, 2 : FW - 2], Alu.is_le
                )

                # q_dir = max(n1+1, n2).  n1 is always the row-1 (or col-1)
                # neighbor, so precompute mag+1 / mp3+1 once (tensor_scalar
                # runs 4x) and use plain TT max ops (2x) for every q.
                mg1 = mid.tile([P, FW], dt.bfloat16, tag="mg1")
                nc.vector.tensor_scalar(mg1[:], mag[:], 1.0, None, Alu.add)
                mp1 = aux.tile([P, SLK], dt.bfloat16, tag="mp1")
                nc.vector.tensor_scalar(mp1[:], mp3[:], 1.0, None, Alu.add)

                q = mid.tile([P, FW], dt.bfloat16, tag="u")  # reuse u slot
                # q := q_d2 (n1=NE=row-1,col+1 ; n2=SW=row+1,col-1)
                nc.vector.tensor_tensor(
                    q[:, 0:SL], mp1[:, 4 : 4 + SL], mag[:, SL - 2 : 2 * SL - 2], Alu.max
                )
                nc.vector.tensor_tensor(
                    q[:, SL : 3 * SL],
                    mg1[:, 2 : 2 * SL + 2],
                    mag[:, 2 * SL - 2 : 4 * SL - 2],
                    Alu.max,
                )
                nc.vector.tensor_tensor(
                    q[:, 3 * SL : FW],
                    mg1[:, 2 * SL + 2 : 3 * SL + 2],
                    mn0[:, 0:SL],
                    Alu.max,
                )
                # q_d1 (n1=NW=row-1,col-1 ; n2=SE=row+1,col+1)
                qd1 = mid.tile([P, FW], dt.bfloat16, tag="gx")  # reuse gx slot
                nc.vector.tensor_tensor(
                    qd1[:, 0 : SL + 2],
                    mp1[:, 0 : SL + 2],
                    mag[:, SL + 2 : 2 * SL + 4],
                    Alu.max,
                )
                nc.vector.tensor_tensor(
                    qd1[:, SL + 2 : 3 * SL - 2],
                    mg1[:, 0 : 2 * SL - 4],
                    mag[:, 2 * SL + 4 : 4 * SL],
                    Alu.max,
                )
                nc.vector.tensor_tensor(
                    qd1[:, 3 * SL - 2 : FW],
                    mg1[:, 2 * SL - 4 : 3 * SL - 2],
                    mn0[:, 2 : SL + 4],
                    Alu.max,
                )
                # q_v (n1=N=row-1 ; n2=S=row+1)
                qv = mid.tile([P, FW], dt.bfloat16, tag="ax")  # reuse ax slot
                nc.vector.tensor_tensor(
                    qv[:, 0:SL], mp1[:, 2 : 2 + SL], mag[:, SL : 2 * SL], Alu.max
                )
                nc.vector.tensor_tensor(
                    qv[:, SL : 3 * SL], mg1[:, 0 : 2 * SL], mag[:, 2 * SL : FW], Alu.max
                )
                nc.vector.tensor_tensor(
                    qv[:, 3 * SL : FW],
                    mg1[:, 2 * SL : 3 * SL],
                    mn0[:, 2 : 2 + SL],
                    Alu.max,
                )
                # q_h (n1=W=col-1 ; n2=E=col+1)
                qh = mid.tile([P, FW], dt.bfloat16, tag="ay")  # reuse ay slot
                nc.vector.tensor_tensor(
                    qh[:, 2 : FW - 2], mg1[:, 0 : FW - 4], mag[:, 4:FW], Alu.max
                )

                # priority select: d2 -> d1 (diag_pos) -> v (is_v) -> h (is_h)
                nc.vector.copy_predicated(
                    q[:, 2 : FW - 2], dp[:, 2 : FW - 2], qd1[:, 2 : FW - 2]
                )
                nc.vector.copy_predicated(
                    q[:, 2 : FW - 2], isv[:, 2 : FW - 2], qv[:, 2 : FW - 2]
                )
                nc.vector.copy_predicated(
                    q[:, 2 : FW - 2], ish[:, 2 : FW - 2], qh[:, 2 : FW - 2]
                )

                keep = mid.tile([P, FW], dt.bfloat16, tag="r")  # reuse r slot
                nc.vector.tensor_tensor(
                    keep[:, 2 : FW - 2], mag[:, 2 : FW - 2], q[:, 2 : FW - 2], Alu.is_ge
                )

                # d = (keep_pred != keep_label), accumulated count per partition
                d = mid.tile([P, J * W], dt.bfloat16, tag="d")
                dv = d[:].rearrange("p (j w e) -> p j w e", j=J, e=1)
                nc.vector.scalar_tensor_tensor(
                    dv,
                    v4(keep)[:, :, 1 : 1 + W, 0:1],
                    1.0,
                    v4(keep)[:, :, 1 : 1 + W, 1:2],
                    Alu.mult,
                    Alu.not_equal,
                    accum_out=acc[:, k : k + 1],
                )

            nc.sync.dma_start(accd[:], acc[:])

    nc.compile()
    return nc


def _get_program():
    if "nc" not in _CACHE:
        _CACHE["nc"] = _build_program()
    return _CACHE["nc"]


def kernel(pred: np.ndarray, labels: np.ndarray) -> np.ndarray:
    from concourse import bass_utils

    pred = np.asarray(pred).reshape(B, H, W).astype(np.float32, copy=False)
    labels = np.asarray(labels).reshape(B, H, W).astype(np.float32, copy=False)

    nc = _get_program()
    shifts = _shift_mats()
    in_maps = []
    for c in range(NCORES):
        sl = slice(c * PAIRS, (c + 1) * PAIRS)
        in_maps.append(
            {
                "pred": np.ascontiguousarray(pred[sl]),
                "labels": np.ascontiguousarray(labels[sl]),
                "shifts": shifts,
            }
        )
    res = bass_utils.run_bass_kernel_spmd(nc, in_maps, core_ids=list(range(NCORES)))
    k_total = sum(float(r["acc_out"].astype(np.float64).sum()) for r in res.results)
    loss = np.float32(_bce_constant() * k_total / float(N_TOT))
    return np.array(loss, dtype=np.float32)
